# revision 30
# baseline (speedup 1.0000x reference)
"""MoE blended-expert MLP (nn_Expert_44538810860342) on 8 trn2 NeuronCores.

Math (per layer): y = sum_e wb[:,e] * (x @ W_e^T) + wb @ B
Rewritten as a single stacked-K GEMM: the blend weight is folded into the
moving operand (per-expert scaled activations), the bias blend is a K=8
matmul into the same PSUM accumulation group, and the ELU's "-1" term is
folded into the next layer's effective bias (B_eff = B - rowsum(W)), so the
device stores s = elu(a) + 1 = relu(a) + exp(min(a, 0)).

Dataflow is feature-major throughout: activations live as [feature, batch]
tiles so the layer output (PSUM [out_feat, batch]) directly feeds the next
layer's moving operand. Data-parallel over batch: each of 8 cores takes 512
rows; the per-expert weight stacks are replicated and streamed from HBM.
Matmuls run as float32r (full PE rate at N=512).
"""

import os

import numpy as np

import concourse.bass as bass
import concourse.tile as tile
from concourse import bacc, mybir
from concourse.bass_utils import run_bass_kernel_spmd

BATCH, E = 4096, 8
DIMS = [512, 1024, 1024, 512]
N_CORES = 8
BC = BATCH // N_CORES  # 512 batch rows per core
F32 = mybir.dt.float32
F32R = mybir.dt.float32r

_cache = {}


def _build():
    """Build + compile the per-core Bass program (identical on all cores)."""
    nc = bacc.Bacc("TRN2", target_bir_lowering=False, debug=False,
                   enable_asserts=False, num_devices=N_CORES)

    xT = nc.dram_tensor("xT", [DIMS[0], BC], F32, kind="ExternalInput").ap()
    wbT = nc.dram_tensor("wbT", [E, BC], F32R, kind="ExternalInput").ap()
    wbB = nc.dram_tensor("wbB", [E, 128, BC], F32, kind="ExternalInput").ap()
    wts = [
        nc.dram_tensor(f"w{l}t", [E, DIMS[l], DIMS[l + 1]], F32R,
                       kind="ExternalInput").ap()
        for l in range(3)
    ]
    bes = [
        nc.dram_tensor(f"b{l}e", [E, DIMS[l + 1]], F32R,
                       kind="ExternalInput").ap()
        for l in range(3)
    ]
    outT = nc.dram_tensor("outT", [DIMS[3], BC], F32, kind="ExternalOutput").ap()

    with tile.TileContext(nc) as tc:
        with (
            tc.tile_pool(name="const", bufs=1) as const,
            tc.tile_pool(name="acts", bufs=1) as acts,
            tc.tile_pool(name="w", bufs=18) as wpool,
            tc.tile_pool(name="rhs", bufs=12) as rhspool,
            tc.tile_pool(name="scr", bufs=4) as scratch,
            tc.tile_pool(name="ps", bufs=8, space="PSUM") as psum,
        ):
            # PE warmup: dependency-free matmuls on a memset tile so the HAM
            # clock gate opens (1.2 -> 2.4 GHz) while the first DMAs land
            warm = const.tile([128, 128], mybir.dt.bfloat16, tag="warm")
            nc.gpsimd.memset(warm[:], 0.0)
            warm_ps = psum.tile([128, BC], F32, tag="ps", name="warm_ps")
            for i in range(32):
                nc.tensor.matmul(warm_ps[:, 0:128], warm[:], warm[:],
                                 start=True, stop=True)

            # --- constants: issue on the Scalar engine's HWDGE queue so they
            # don't serialize ahead of the first weight/x DMAs on Sync ---
            wbT_sb = const.tile([E, BC], F32R, tag="wbT")
            nc.sync.dma_start(wbT_sb[:], wbT[:])

            bias_sb = []
            for l in range(3):
                t = const.tile([E, DIMS[l + 1]], F32R, tag=f"b{l}",
                               name=f"bias{l}")
                (nc.sync if l == 0 else nc.gpsimd).dma_start(t[:], bes[l][:])
                bias_sb.append(t)

            # input activations, feature-major [feat, batch] as 128-row tiles.
            # x0 + blend broadcasts lead the sync queue (they gate the first
            # scaled-operand mul); the weight stream follows behind them.
            x_sb = acts.tile([128, DIMS[0] // 128, BC], F32, tag="x")
            nc.sync.dma_start(x_sb[:, 0, :], xT[0:128, :])
            wb_bc = const.tile([128, E, BC], F32, tag="wbB")
            nc.sync.dma_start(wb_bc[:, 0, :], wbB[0])
            for k in range(1, DIMS[0] // 128):
                nc.scalar.dma_start(
                    x_sb[:, k, :], xT[k * 128:(k + 1) * 128, :])

            h_sb = [x_sb]
            for l in range(3):
                t = acts.tile([128, DIMS[l + 1] // 128, BC], F32, tag=f"h{l}")
                h_sb.append(t)

            premade_rhs = {}

            # --- layers (k-outer: each finished h-tile feeds nm*E matmuls
            # while the ELU chain produces the next one; the final k-tile is
            # emitted bank-major so banks complete staggered and the next
            # layer never waits on a full-width epilogue) ---
            for l in range(3):
                d_in, d_out = DIMS[l], DIMS[l + 1]
                nk, nm = d_in // 128, d_out // 128
                hin, hout = h_sb[l], h_sb[l + 1]

                ps = [psum.tile([128, BC], F32, tag="ps", name=f"ps{l}_{m}")
                      for m in range(nm)]
                # bias blend: psum[m] = B_eff[:, m].T @ wbT  (K=8, starts group)
                for m in range(nm):
                    nc.tensor.matmul(
                        ps[m][:],
                        bias_sb[l][:, m * 128:(m + 1) * 128],
                        wbT_sb[:],
                        start=True, stop=False,
                    )

                def epilogue(m, ps=ps, hout=hout, l=l, nm=nm):
                    if l < 2:
                        # After bank m's ELU, immediately produce the next
                        # layer's (k=0, e=m) scaled operand so the first
                        # next-layer matmul isn't queued behind the full
                        # epilogue in the DVE FIFO.
                        # s = relu(a) + min(exp(a), 1) = elu(a) + 1
                        # (Exp saturates to inf/0 on ACT — probed, no NaN)
                        rt = scratch.tile([128, BC], F32, tag="rt",
                                          name=f"relu{l}_{m}")
                        nc.scalar.activation(
                            rt[:], ps[m][:], mybir.ActivationFunctionType.Relu)
                        et = scratch.tile([128, BC], F32, tag="et",
                                          name=f"exp{l}_{m}")
                        nc.scalar.activation(
                            et[:], ps[m][:], mybir.ActivationFunctionType.Exp)
                        nc.vector.scalar_tensor_tensor(
                            hout[:, m, :], et[:], 1.0, rt[:],
                            mybir.AluOpType.min, mybir.AluOpType.add)
                        if m < E:
                            nrhs = rhspool.tile([128, BC], F32R, tag="rhs",
                                                name=f"rhs{l + 1}_0_{m}")
                            nc.vector.tensor_mul(
                                nrhs[:], hout[:, 0, :], wb_bc[:, m, :])
                            premade_rhs[(l + 1, m)] = nrhs
                    else:
                        ot = scratch.tile([128, BC], F32, tag="ot",
                                          name=f"out{m}", bufs=2)
                        nc.vector.tensor_copy(ot[:], ps[m][:])
                        nc.sync.dma_start(outT[m * 128:(m + 1) * 128, :], ot[:])

                for k in range(nk):
                    ke_tiles = []
                    for e in range(E):
                        wtg = wpool.tile([128, 1, d_out], F32R, tag="w",
                                         name=f"w{l}_{k}_{e}")
                        half = d_out // 2
                        nc.sync.dma_start(
                            wtg[:, 0, 0:half],
                            wts[l][e, k * 128:(k + 1) * 128, 0:half])
                        nc.scalar.dma_start(
                            wtg[:, 0, half:d_out],
                            wts[l][e, k * 128:(k + 1) * 128, half:d_out])
                        if l == 0 and k == 0 and e < E - 1:
                            nc.sync.dma_start(wb_bc[:, e + 1, :], wbB[e + 1])
                        rhs_t = premade_rhs.pop((l, e), None) if k == 0 else None
                        if rhs_t is None:
                            rhs_t = rhspool.tile([128, BC], F32R, tag="rhs",
                                                 name=f"rhs{l}_{k}_{e}")
                            nc.vector.tensor_mul(
                                rhs_t[:], hin[:, k, :], wb_bc[:, e, :])
                        ke_tiles.append((wtg, rhs_t))
                    kk = 0
                    if k < nk - 1:
                        for e in range(E):
                            wtg, rhs_t = ke_tiles[e]
                            for m in range(nm):
                                nc.tensor.matmul(
                                    ps[m][:], wtg[:, kk, m * 128:(m + 1) * 128],
                                    rhs_t[:], start=False, stop=False)
                    else:
                        # final k-tile: bank-major + staggered epilogue
                        for m in range(nm):
                            for e in range(E):
                                wtg, rhs_t = ke_tiles[e]
                                nc.tensor.matmul(
                                    ps[m][:], wtg[:, kk, m * 128:(m + 1) * 128],
                                    rhs_t[:], start=False, stop=(e == E - 1))
                            epilogue(m)

    nc.compile()
    return nc


def kernel(weight_blend, x, W0, B0, W1, B1, W2, B2):
    wb = np.asarray(weight_blend, dtype=np.float32)
    x = np.asarray(x, dtype=np.float32)
    Ws = [np.asarray(W, dtype=np.float32) for W in (W0, W1, W2)]
    Bs = [np.asarray(B, dtype=np.float32) for B in (B0, B1, B2)]

    if "nc" not in _cache:
        _cache["nc"] = _build()
    nc = _cache["nc"]

    # host-side layout prep (not on the device critical path)
    wts = [np.ascontiguousarray(W.transpose(0, 2, 1)) for W in Ws]
    bes = [Bs[0]]
    for l in (1, 2):
        # next-layer input is stored as elu+1; fold the -1 via rowsums
        bes.append(np.ascontiguousarray(Bs[l] - Ws[l].sum(axis=2)))

    in_maps = []
    for c in range(N_CORES):
        sl = slice(c * BC, (c + 1) * BC)
        wbT = np.ascontiguousarray(wb[sl].T)
        m = {
            "xT": np.ascontiguousarray(x[sl].T),
            "wbT": wbT,
            "wbB": np.ascontiguousarray(
                np.broadcast_to(wbT[:, None, :], (E, 128, BC))),
            "w0t": wts[0], "w1t": wts[1], "w2t": wts[2],
            "b0e": bes[0], "b1e": bes[1], "b2e": bes[2],
        }
        in_maps.append(m)

    res = run_bass_kernel_spmd(nc, in_maps, core_ids=list(range(N_CORES)))
    kernel._last_results = res

    out = np.empty((BATCH, DIMS[3]), dtype=np.float32)
    for c in range(N_CORES):
        out[c * BC:(c + 1) * BC] = res.results[c]["outT"].T
    return out


# revision 31
# speedup vs baseline: 1.0660x; 1.0660x over previous
"""MoE blended-expert MLP (nn_Expert_44538810860342) on 8 trn2 NeuronCores.

Math (per layer): y = sum_e wb[:,e] * (x @ W_e^T) + wb @ B
Rewritten as a single stacked-K GEMM: the blend weight is folded into the
moving operand (per-expert scaled activations), the bias blend is a K=8
matmul into the same PSUM accumulation group, and the ELU's "-1" term is
folded into the next layer's effective bias (B_eff = B - rowsum(W)), so the
device stores s = elu(a) + 1 = relu(a) + exp(min(a, 0)).

Dataflow is feature-major throughout: activations live as [feature, batch]
tiles so the layer output (PSUM [out_feat, batch]) directly feeds the next
layer's moving operand. Data-parallel over batch: each of 8 cores takes 512
rows; the per-expert weight stacks are replicated and streamed from HBM.
Matmuls run as float32r (full PE rate at N=512).
"""

import os

import numpy as np

import concourse.bass as bass
import concourse.tile as tile
from concourse import bacc, mybir
from concourse.bass_utils import run_bass_kernel_spmd

BATCH, E = 4096, 8
DIMS = [512, 1024, 1024, 512]
N_CORES = 8
BC = BATCH // N_CORES  # 512 batch rows per core
F32 = mybir.dt.float32
F32R = mybir.dt.float32r

_cache = {}


def _build():
    """Build + compile the per-core Bass program (identical on all cores)."""
    nc = bacc.Bacc("TRN2", target_bir_lowering=False, debug=False,
                   enable_asserts=False, num_devices=N_CORES)

    xT = nc.dram_tensor("xT", [DIMS[0], BC], F32, kind="ExternalInput").ap()
    wbT = nc.dram_tensor("wbT", [E, BC], F32R, kind="ExternalInput").ap()
    wbB = nc.dram_tensor("wbB", [E, 128, BC], F32, kind="ExternalInput").ap()
    wts = [
        nc.dram_tensor(f"w{l}t", [E, DIMS[l], DIMS[l + 1]], F32R,
                       kind="ExternalInput").ap()
        for l in range(3)
    ]
    bes = [
        nc.dram_tensor(f"b{l}e", [E, DIMS[l + 1]], F32R,
                       kind="ExternalInput").ap()
        for l in range(3)
    ]
    outT = nc.dram_tensor("outT", [DIMS[3], BC], F32, kind="ExternalOutput").ap()

    with tile.TileContext(nc) as tc:
        with (
            tc.tile_pool(name="const", bufs=1) as const,
            tc.tile_pool(name="acts", bufs=1) as acts,
            tc.tile_pool(name="w", bufs=18) as wpool,
            tc.tile_pool(name="rhs", bufs=12) as rhspool,
            tc.tile_pool(name="scr", bufs=4) as scratch,
            tc.tile_pool(name="ps", bufs=8, space="PSUM") as psum,
        ):
            # PE warmup: dependency-free matmuls on a memset tile so the HAM
            # clock gate opens (1.2 -> 2.4 GHz) while the first DMAs land
            warm = const.tile([128, 128], mybir.dt.bfloat16, tag="warm")
            nc.gpsimd.memset(warm[:], 0.0)
            warm_ps = psum.tile([128, BC], F32, tag="ps", name="warm_ps")
            for i in range(32):
                nc.tensor.matmul(warm_ps[:, 0:128], warm[:], warm[:],
                                 start=True, stop=True)

            # --- constants: issue on the Scalar engine's HWDGE queue so they
            # don't serialize ahead of the first weight/x DMAs on Sync ---
            wbT_sb = const.tile([E, BC], F32R, tag="wbT")
            nc.sync.dma_start(wbT_sb[:], wbT[:])

            bias_sb = []
            for l in range(3):
                t = const.tile([E, DIMS[l + 1]], F32R, tag=f"b{l}",
                               name=f"bias{l}")
                (nc.sync if l == 0 else nc.gpsimd).dma_start(t[:], bes[l][:])
                bias_sb.append(t)

            # input activations, feature-major [feat, batch] as 128-row tiles.
            # x0 + blend broadcasts lead the sync queue (they gate the first
            # scaled-operand mul); the weight stream follows behind them.
            x_sb = acts.tile([128, DIMS[0] // 128, BC], F32, tag="x")
            nc.sync.dma_start(x_sb[:, 0, :], xT[0:128, :])
            wb_bc = const.tile([128, E, BC], F32, tag="wbB")
            nc.sync.dma_start(wb_bc[:, 0, :], wbB[0])
            for k in range(1, DIMS[0] // 128):
                nc.scalar.dma_start(
                    x_sb[:, k, :], xT[k * 128:(k + 1) * 128, :])

            h_sb = [x_sb]
            for l in range(3):
                t = acts.tile([128, DIMS[l + 1] // 128, BC], F32, tag=f"h{l}")
                h_sb.append(t)

            premade_rhs = {}

            # --- layers (k-outer: each finished h-tile feeds nm*E matmuls
            # while the ELU chain produces the next one; the final k-tile is
            # emitted bank-major so banks complete staggered and the next
            # layer never waits on a full-width epilogue) ---
            for l in range(3):
                d_in, d_out = DIMS[l], DIMS[l + 1]
                nk, nm = d_in // 128, d_out // 128
                hin, hout = h_sb[l], h_sb[l + 1]

                ps = [psum.tile([128, BC], F32, tag="ps", name=f"ps{l}_{m}")
                      for m in range(nm)]
                # bias blend: psum[m] = B_eff[:, m].T @ wbT  (K=8, starts group)
                for m in range(nm):
                    nc.tensor.matmul(
                        ps[m][:],
                        bias_sb[l][:, m * 128:(m + 1) * 128],
                        wbT_sb[:],
                        start=True, stop=False,
                    )

                def epilogue(m, ps=ps, hout=hout, l=l, nm=nm):
                    if l < 2:
                        # After bank m's ELU, immediately produce the next
                        # layer's (k=0, e=m) scaled operand so the first
                        # next-layer matmul isn't queued behind the full
                        # epilogue in the DVE FIFO.
                        # s = relu(a) + min(exp(a), 1) = elu(a) + 1
                        # (Exp saturates to inf/0 on ACT — probed, no NaN)
                        rt = scratch.tile([128, BC], F32, tag="rt",
                                          name=f"relu{l}_{m}")
                        nc.scalar.activation(
                            rt[:], ps[m][:], mybir.ActivationFunctionType.Relu)
                        et = scratch.tile([128, BC], F32, tag="et",
                                          name=f"exp{l}_{m}")
                        nc.scalar.activation(
                            et[:], ps[m][:], mybir.ActivationFunctionType.Exp)
                        nc.vector.scalar_tensor_tensor(
                            hout[:, m, :], et[:], 1.0, rt[:],
                            mybir.AluOpType.min, mybir.AluOpType.add)
                        if m < E:
                            nrhs = rhspool.tile([128, BC], F32R, tag="rhs",
                                                name=f"rhs{l + 1}_0_{m}")
                            nc.vector.tensor_mul(
                                nrhs[:], hout[:, 0, :], wb_bc[:, m, :])
                            premade_rhs[(l + 1, m)] = nrhs
                    else:
                        ot = scratch.tile([128, BC], F32, tag="ot",
                                          name=f"out{m}", bufs=2)
                        nc.vector.tensor_copy(ot[:], ps[m][:])
                        nc.sync.dma_start(outT[m * 128:(m + 1) * 128, :], ot[:])

                for k in range(nk):
                    ke_tiles = []
                    for e in range(E):
                        wtg = wpool.tile([128, 1, d_out], F32R, tag="w",
                                         name=f"w{l}_{k}_{e}")
                        nc.sync.dma_start(
                            wtg[:, 0, :], wts[l][e, k * 128:(k + 1) * 128, :])
                        if l == 0 and k == 0 and e < E - 1:
                            nc.sync.dma_start(wb_bc[:, e + 1, :], wbB[e + 1])
                        rhs_t = premade_rhs.pop((l, e), None) if k == 0 else None
                        if rhs_t is None:
                            rhs_t = rhspool.tile([128, BC], F32R, tag="rhs",
                                                 name=f"rhs{l}_{k}_{e}")
                            nc.vector.tensor_mul(
                                rhs_t[:], hin[:, k, :], wb_bc[:, e, :])
                        ke_tiles.append((wtg, rhs_t))
                    kk = 0
                    if k < nk - 1:
                        for e in range(E):
                            wtg, rhs_t = ke_tiles[e]
                            for m in range(nm):
                                nc.tensor.matmul(
                                    ps[m][:], wtg[:, kk, m * 128:(m + 1) * 128],
                                    rhs_t[:], start=False, stop=False)
                    else:
                        # final k-tile: bank-major + staggered epilogue
                        for m in range(nm):
                            for e in range(E):
                                wtg, rhs_t = ke_tiles[e]
                                nc.tensor.matmul(
                                    ps[m][:], wtg[:, kk, m * 128:(m + 1) * 128],
                                    rhs_t[:], start=False, stop=(e == E - 1))
                            epilogue(m)

    nc.compile()
    return nc


def kernel(weight_blend, x, W0, B0, W1, B1, W2, B2):
    wb = np.asarray(weight_blend, dtype=np.float32)
    x = np.asarray(x, dtype=np.float32)
    Ws = [np.asarray(W, dtype=np.float32) for W in (W0, W1, W2)]
    Bs = [np.asarray(B, dtype=np.float32) for B in (B0, B1, B2)]

    if "nc" not in _cache:
        _cache["nc"] = _build()
    nc = _cache["nc"]

    # host-side layout prep (not on the device critical path)
    wts = [np.ascontiguousarray(W.transpose(0, 2, 1)) for W in Ws]
    bes = [Bs[0]]
    for l in (1, 2):
        # next-layer input is stored as elu+1; fold the -1 via rowsums
        bes.append(np.ascontiguousarray(Bs[l] - Ws[l].sum(axis=2)))

    in_maps = []
    for c in range(N_CORES):
        sl = slice(c * BC, (c + 1) * BC)
        wbT = np.ascontiguousarray(wb[sl].T)
        m = {
            "xT": np.ascontiguousarray(x[sl].T),
            "wbT": wbT,
            "wbB": np.ascontiguousarray(
                np.broadcast_to(wbT[:, None, :], (E, 128, BC))),
            "w0t": wts[0], "w1t": wts[1], "w2t": wts[2],
            "b0e": bes[0], "b1e": bes[1], "b2e": bes[2],
        }
        in_maps.append(m)

    res = run_bass_kernel_spmd(nc, in_maps, core_ids=list(range(N_CORES)))
    kernel._last_results = res

    out = np.empty((BATCH, DIMS[3]), dtype=np.float32)
    for c in range(N_CORES):
        out[c * BC:(c + 1) * BC] = res.results[c]["outT"].T
    return out


# revision 32
# speedup vs baseline: 1.0762x; 1.0096x over previous
"""MoE blended-expert MLP (nn_Expert_44538810860342) on 8 trn2 NeuronCores.

Math (per layer): y = sum_e wb[:,e] * (x @ W_e^T) + wb @ B
Rewritten as a single stacked-K GEMM: the blend weight is folded into the
moving operand (per-expert scaled activations), the bias blend is a K=8
matmul into the same PSUM accumulation group, and the ELU's "-1" term is
folded into the next layer's effective bias (B_eff = B - rowsum(W)), so the
device stores s = elu(a) + 1 = relu(a) + exp(min(a, 0)).

Dataflow is feature-major throughout: activations live as [feature, batch]
tiles so the layer output (PSUM [out_feat, batch]) directly feeds the next
layer's moving operand. Data-parallel over batch: each of 8 cores takes 512
rows; the per-expert weight stacks are replicated and streamed from HBM.
Matmuls run as float32r (full PE rate at N=512).
"""

import os

import numpy as np

import concourse.bass as bass
import concourse.tile as tile
from concourse import bacc, mybir
from concourse.bass_utils import run_bass_kernel_spmd

BATCH, E = 4096, 8
DIMS = [512, 1024, 1024, 512]
N_CORES = 8
BC = BATCH // N_CORES  # 512 batch rows per core
F32 = mybir.dt.float32
F32R = mybir.dt.float32r

_cache = {}


def _build():
    """Build + compile the per-core Bass program (identical on all cores)."""
    nc = bacc.Bacc("TRN2", target_bir_lowering=False, debug=False,
                   enable_asserts=False, num_devices=N_CORES)

    xT = nc.dram_tensor("xT", [DIMS[0], BC], F32, kind="ExternalInput").ap()
    wbT = nc.dram_tensor("wbT", [E, BC], F32R, kind="ExternalInput").ap()
    wbB = nc.dram_tensor("wbB", [E, 128, BC], F32, kind="ExternalInput").ap()
    wts = [
        nc.dram_tensor(f"w{l}t", [E, DIMS[l], DIMS[l + 1]], F32R,
                       kind="ExternalInput").ap()
        for l in range(3)
    ]
    bes = [
        nc.dram_tensor(f"b{l}e", [E, DIMS[l + 1]], F32R,
                       kind="ExternalInput").ap()
        for l in range(3)
    ]
    outT = nc.dram_tensor("outT", [DIMS[3], BC], F32, kind="ExternalOutput").ap()

    with tile.TileContext(nc) as tc:
        with (
            tc.tile_pool(name="const", bufs=1) as const,
            tc.tile_pool(name="acts", bufs=1) as acts,
            tc.tile_pool(name="w", bufs=18) as wpool,
            tc.tile_pool(name="rhs", bufs=12) as rhspool,
            tc.tile_pool(name="scr", bufs=4) as scratch,
            tc.tile_pool(name="ps", bufs=8, space="PSUM") as psum,
        ):
            # PE warmup: dependency-free matmuls on a memset tile so the HAM
            # clock gate opens (1.2 -> 2.4 GHz) while the first DMAs land
            warm = const.tile([128, 128], mybir.dt.bfloat16, tag="warm")
            nc.gpsimd.memset(warm[:], 0.0)
            warm_ps = psum.tile([128, BC], F32, tag="ps", name="warm_ps")
            for i in range(32):
                nc.tensor.matmul(warm_ps[:, 0:128], warm[:], warm[:],
                                 start=True, stop=True)

            # --- constants: issue on the Scalar engine's HWDGE queue so they
            # don't serialize ahead of the first weight/x DMAs on Sync ---
            wbT_sb = const.tile([E, BC], F32R, tag="wbT")
            nc.sync.dma_start(wbT_sb[:], wbT[:])

            bias_sb = []
            for l in range(3):
                t = const.tile([E, DIMS[l + 1]], F32R, tag=f"b{l}",
                               name=f"bias{l}")
                (nc.sync if l == 0 else nc.gpsimd).dma_start(t[:], bes[l][:])
                bias_sb.append(t)

            # input activations, feature-major [feat, batch] as 128-row tiles.
            # x0 + blend broadcasts lead the sync queue (they gate the first
            # scaled-operand mul); the weight stream follows behind them.
            x_sb = acts.tile([128, DIMS[0] // 128, BC], F32, tag="x")
            nc.sync.dma_start(x_sb[:, 0, :], xT[0:128, :])
            wb_bc = const.tile([128, E, BC], F32, tag="wbB")
            nc.scalar.dma_start(wb_bc[:, 0, :], wbB[0])
            for k in range(1, DIMS[0] // 128):
                nc.scalar.dma_start(
                    x_sb[:, k, :], xT[k * 128:(k + 1) * 128, :])

            h_sb = [x_sb]
            for l in range(3):
                t = acts.tile([128, DIMS[l + 1] // 128, BC], F32, tag=f"h{l}")
                h_sb.append(t)

            premade_rhs = {}

            # --- layers (k-outer: each finished h-tile feeds nm*E matmuls
            # while the ELU chain produces the next one; the final k-tile is
            # emitted bank-major so banks complete staggered and the next
            # layer never waits on a full-width epilogue) ---
            for l in range(3):
                d_in, d_out = DIMS[l], DIMS[l + 1]
                nk, nm = d_in // 128, d_out // 128
                hin, hout = h_sb[l], h_sb[l + 1]

                ps = [psum.tile([128, BC], F32, tag="ps", name=f"ps{l}_{m}")
                      for m in range(nm)]
                # bias blend: psum[m] = B_eff[:, m].T @ wbT  (K=8, starts group)
                for m in range(nm):
                    nc.tensor.matmul(
                        ps[m][:],
                        bias_sb[l][:, m * 128:(m + 1) * 128],
                        wbT_sb[:],
                        start=True, stop=False,
                    )

                def epilogue(m, ps=ps, hout=hout, l=l, nm=nm):
                    if l < 2:
                        # After bank m's ELU, immediately produce the next
                        # layer's (k=0, e=m) scaled operand so the first
                        # next-layer matmul isn't queued behind the full
                        # epilogue in the DVE FIFO.
                        # s = relu(a) + min(exp(a), 1) = elu(a) + 1
                        # (Exp saturates to inf/0 on ACT — probed, no NaN)
                        rt = scratch.tile([128, BC], F32, tag="rt",
                                          name=f"relu{l}_{m}")
                        nc.scalar.activation(
                            rt[:], ps[m][:], mybir.ActivationFunctionType.Relu)
                        et = scratch.tile([128, BC], F32, tag="et",
                                          name=f"exp{l}_{m}")
                        nc.scalar.activation(
                            et[:], ps[m][:], mybir.ActivationFunctionType.Exp)
                        nc.vector.scalar_tensor_tensor(
                            hout[:, m, :], et[:], 1.0, rt[:],
                            mybir.AluOpType.min, mybir.AluOpType.add)
                        if m < E:
                            nrhs = rhspool.tile([128, BC], F32R, tag="rhs",
                                                name=f"rhs{l + 1}_0_{m}")
                            nc.vector.tensor_mul(
                                nrhs[:], hout[:, 0, :], wb_bc[:, m, :])
                            premade_rhs[(l + 1, m)] = nrhs
                    else:
                        ot = scratch.tile([128, BC], F32, tag="ot",
                                          name=f"out{m}", bufs=2)
                        nc.vector.tensor_copy(ot[:], ps[m][:])
                        nc.sync.dma_start(outT[m * 128:(m + 1) * 128, :], ot[:])

                for k in range(nk):
                    ke_tiles = []
                    for e in range(E):
                        wtg = wpool.tile([128, 1, d_out], F32R, tag="w",
                                         name=f"w{l}_{k}_{e}")
                        nc.sync.dma_start(
                            wtg[:, 0, :], wts[l][e, k * 128:(k + 1) * 128, :])
                        if l == 0 and k == 0 and e < E - 1:
                            nc.sync.dma_start(wb_bc[:, e + 1, :], wbB[e + 1])
                        rhs_t = premade_rhs.pop((l, e), None) if k == 0 else None
                        if rhs_t is None:
                            rhs_t = rhspool.tile([128, BC], F32R, tag="rhs",
                                                 name=f"rhs{l}_{k}_{e}")
                            nc.vector.tensor_mul(
                                rhs_t[:], hin[:, k, :], wb_bc[:, e, :])
                        ke_tiles.append((wtg, rhs_t))
                    kk = 0
                    if k < nk - 1:
                        for e in range(E):
                            wtg, rhs_t = ke_tiles[e]
                            for m in range(nm):
                                nc.tensor.matmul(
                                    ps[m][:], wtg[:, kk, m * 128:(m + 1) * 128],
                                    rhs_t[:], start=False, stop=False)
                    else:
                        # final k-tile: bank-major + staggered epilogue
                        for m in range(nm):
                            for e in range(E):
                                wtg, rhs_t = ke_tiles[e]
                                nc.tensor.matmul(
                                    ps[m][:], wtg[:, kk, m * 128:(m + 1) * 128],
                                    rhs_t[:], start=False, stop=(e == E - 1))
                            epilogue(m)

    nc.compile()
    return nc


def kernel(weight_blend, x, W0, B0, W1, B1, W2, B2):
    wb = np.asarray(weight_blend, dtype=np.float32)
    x = np.asarray(x, dtype=np.float32)
    Ws = [np.asarray(W, dtype=np.float32) for W in (W0, W1, W2)]
    Bs = [np.asarray(B, dtype=np.float32) for B in (B0, B1, B2)]

    if "nc" not in _cache:
        _cache["nc"] = _build()
    nc = _cache["nc"]

    # host-side layout prep (not on the device critical path)
    wts = [np.ascontiguousarray(W.transpose(0, 2, 1)) for W in Ws]
    bes = [Bs[0]]
    for l in (1, 2):
        # next-layer input is stored as elu+1; fold the -1 via rowsums
        bes.append(np.ascontiguousarray(Bs[l] - Ws[l].sum(axis=2)))

    in_maps = []
    for c in range(N_CORES):
        sl = slice(c * BC, (c + 1) * BC)
        wbT = np.ascontiguousarray(wb[sl].T)
        m = {
            "xT": np.ascontiguousarray(x[sl].T),
            "wbT": wbT,
            "wbB": np.ascontiguousarray(
                np.broadcast_to(wbT[:, None, :], (E, 128, BC))),
            "w0t": wts[0], "w1t": wts[1], "w2t": wts[2],
            "b0e": bes[0], "b1e": bes[1], "b2e": bes[2],
        }
        in_maps.append(m)

    res = run_bass_kernel_spmd(nc, in_maps, core_ids=list(range(N_CORES)))
    kernel._last_results = res

    out = np.empty((BATCH, DIMS[3]), dtype=np.float32)
    for c in range(N_CORES):
        out[c * BC:(c + 1) * BC] = res.results[c]["outT"].T
    return out
